# revision 1
# baseline (speedup 1.0000x reference)
"""3x3 stride-2 VALID avg-pool over (8, 64, 512, 512) fp32 on 8 trn2 cores.

Sharding: data-parallel over batch — core i handles x[i] (64 planes of
512x512, contiguous 64 MiB slab). No communication.

Per-core dataflow (planes processed in pairs to halve PE weight-loads):
  1. DMA one plane (1 MiB, contiguous) into SBUF as [128p, 4r, 512w]
     (row h = 4p + r).
  2. DVE W-pool via strided views:  rp[p,r,j] = x[h,2j]+x[h,2j+1]+x[h,2j+2]
     (2 tensor_add ops per plane over [128, 4, 255]).
  3. PE H-pool as a sparse pooling-matrix matmul, two planes packed into
     one moving operand (N = 2*255 = 510 <= 512 fp32 limit): for each
     output-row chunk mc, accumulate over r:
        psum[m, (cc,j)] += mt[:, mc, r, :].T @ rp[:, r, (cc,j)]
     where mt[k, mc, r, m] = 1 iff input row (4k+r) is in the 3-row window
     of output row (mc*128+m).
  4. ScalarE copy PSUM -> SBUF with scale 1/9.
  5. Batched DMA out (4 planes per store pair, 2 stores per group).
"""

import sys

sys.path.insert(0, "/opt/trn_rl_repo")

import numpy as np

from concourse import bacc, bass, mybir, tile
from concourse.bass_utils import run_bass_kernel_spmd

P = 128
B, C, H, W = 8, 64, 512, 512
KS, ST = 3, 2
HO = (H - KS) // ST + 1  # 255
WO = (W - KS) // ST + 1  # 255
CPC = C  # planes per core (one batch image per core)
GROUP = 4  # planes per output-DMA batch
N_CORES = 8

_F32 = mybir.dt.float32


def _pool_matrices() -> np.ndarray:
    """mt[k, mc, r, m] = 1 iff row h=4k+r feeds output row i=mc*128+m."""
    mt = np.zeros((P, 2, 4, P), np.float32)
    k = np.arange(P)[:, None, None, None]
    mc = np.arange(2)[None, :, None, None]
    r = np.arange(4)[None, None, :, None]
    m = np.arange(P)[None, None, None, :]
    h = 4 * k + r
    i = mc * P + m
    mt[(i < HO) & (2 * i <= h) & (h <= 2 * i + 2)] = 1.0
    return mt


def _build_nc(repeat: int = 1) -> bass.Bass:
    nc = bacc.Bacc(None)
    x = nc.declare_dram_parameter("x", [CPC, H, W], _F32, isOutput=False)
    mt = nc.declare_dram_parameter("mt", [P, 2, 4, P], _F32, isOutput=False)
    out = nc.declare_dram_parameter("out", [CPC, HO, WO], _F32, isOutput=True)

    with tile.TileContext(nc) as tc:
        with (
            tc.tile_pool(name="const", bufs=1) as constp,
            tc.tile_pool(name="xin", bufs=8) as xp,
            tc.tile_pool(name="rp", bufs=4) as rpp,
            tc.tile_pool(name="ob", bufs=3) as obp,
            tc.tile_pool(name="ps", bufs=8, space="PSUM") as psp,
        ):
            mt_sb = constp.tile([P, 2, 4, P], _F32)
            nc.sync.dma_start(out=mt_sb[:], in_=mt[:])

            def body():
                for g in range(CPC // GROUP):
                    ob = obp.tile([P, 2, GROUP, WO], _F32)  # [p, chunk, cc, j]
                    for cc in range(GROUP):
                        c = g * GROUP + cc
                        xt = xp.tile([P, 4, W], _F32)
                        nc.sync.dma_start(
                            out=xt[:],
                            in_=x[c].rearrange("(p r) w -> p r w", p=P),
                        )
                        rp = rpp.tile([P, 4, WO], _F32)
                        nc.vector.tensor_add(
                            rp[:],
                            xt[:, :, 0 : 2 * WO : 2],
                            xt[:, :, 1 : 2 * WO + 1 : 2],
                        )
                        nc.vector.tensor_add(
                            rp[:], rp[:], xt[:, :, 2 : 2 * WO + 2 : 2]
                        )
                        for mc in range(2):
                            pst = psp.tile([P, WO], _F32)
                            for r in range(4):
                                nc.tensor.matmul(
                                    pst[:],
                                    mt_sb[:, mc, r, :],
                                    rp[:, r, :],
                                    start=(r == 0),
                                    stop=(r == 3),
                                )
                            nc.scalar.mul(
                                ob[:, mc, cc, :],
                                pst[:],
                                1.0 / 9.0,
                            )
                    og = out[g * GROUP : (g + 1) * GROUP]  # [GROUP, HO, WO]
                    nc.sync.dma_start(
                        out=og[:, 0:P, :].transpose([1, 0, 2]),
                        in_=ob[:, 0, :, :],
                    )
                    nc.sync.dma_start(
                        out=og[:, P:HO, :].transpose([1, 0, 2]),
                        in_=ob[0 : HO - P, 1, :, :],
                    )

            if repeat == 1:
                body()
            else:
                with tc.For_i(0, repeat, 1):
                    body()
    nc.compile()
    return nc


_NC_CACHE: dict = {}


def _get_nc(repeat: int = 1):
    if repeat not in _NC_CACHE:
        _NC_CACHE[repeat] = _build_nc(repeat)
    return _NC_CACHE[repeat]


def kernel(x: np.ndarray, **_unused) -> np.ndarray:
    assert x.shape == (B, C, H, W), x.shape
    x = np.ascontiguousarray(np.asarray(x, dtype=np.float32))
    mt = _pool_matrices()
    in_maps = [{"x": x[i], "mt": mt} for i in range(N_CORES)]
    res = run_bass_kernel_spmd(_get_nc(), in_maps, list(range(N_CORES)))
    return np.stack([res.results[i]["out"] for i in range(N_CORES)], axis=0)



# revision 2
# speedup vs baseline: 1.0735x; 1.0735x over previous
"""3x3 stride-2 VALID avg-pool over (8, 64, 512, 512) fp32 on 8 trn2 cores.

Sharding: data-parallel over batch — core i handles x[i] (64 planes of
512x512, contiguous 64 MiB slab). No communication.

Per-core dataflow (plane layout h = 4p + r, so output row 2p is entirely
within partition p and output row 2p+1 needs one row from partition p+1):
  1. DMA one plane (1 MiB, contiguous) into SBUF as [128p, 4r, 512w].
  2. DVE W-pool via strided views (2 tensor_adds over [128, 4, 255]):
       rp[p,r,j] = x[h,2j] + x[h,2j+1] + x[h,2j+2]
  3. GpSimd H-pool for even output rows (2 tensor_adds over [128, 255]):
       ev[p,j] = rp[p,0,j] + rp[p,1,j] + rp[p,2,j]        (= row 2p sum)
  4. PE H-pool for odd output rows (partition +1 shift is only legal on
     PE; 3 accumulating matmuls, N=255 fp32):
       psum[m,j] = rp[m+1,0,j] + rp[m,2,j] + rp[m,3,j]    (= row 2m+1 sum)
     with stationaries S (subdiagonal) and I (identity).
  5. Act scale by 1/9: ob_even <- ev (SBUF), ob_odd <- psum (PSUM).
  6. Output DMAs issued from the Act queue (keeps the SP queue free for
     input prefetch), batched 4 planes per pair of stores.
"""

import sys

sys.path.insert(0, "/opt/trn_rl_repo")

import numpy as np

from concourse import bacc, bass, mybir, tile
from concourse.bass_utils import run_bass_kernel_spmd

P = 128
B, C, H, W = 8, 64, 512, 512
KS, ST = 3, 2
HO = (H - KS) // ST + 1  # 255
WO = (W - KS) // ST + 1  # 255
CPC = C  # planes per core (one batch image per core)
GROUP = 4  # planes per output-DMA batch
N_CORES = 8

_F32 = mybir.dt.float32


def _shift_mats() -> np.ndarray:
    """T[:,0,:] = S (S[k,m]=1 iff k=m+1), T[:,1,:] = I."""
    t = np.zeros((P, 2, P), np.float32)
    k = np.arange(P)
    t[k[1:], 0, k[:-1]] = 1.0
    t[k, 1, k] = 1.0
    return t


def _build_nc(repeat: int = 1) -> bass.Bass:
    nc = bacc.Bacc(None)
    x = nc.declare_dram_parameter("x", [CPC, H, W], _F32, isOutput=False)
    tmat = nc.declare_dram_parameter("tmat", [P, 2, P], _F32, isOutput=False)
    out = nc.declare_dram_parameter("out", [CPC, HO, WO], _F32, isOutput=True)

    with tile.TileContext(nc) as tc:
        with (
            tc.tile_pool(name="const", bufs=1) as constp,
            tc.tile_pool(name="xin", bufs=8) as xp,
            tc.tile_pool(name="rp", bufs=4) as rpp,
            tc.tile_pool(name="ev", bufs=4) as evp,
            tc.tile_pool(name="ob", bufs=3) as obp,
            tc.tile_pool(name="ps", bufs=4, space="PSUM") as psp,
        ):
            t_sb = constp.tile([P, 2, P], _F32)
            nc.sync.dma_start(out=t_sb[:], in_=tmat[:])

            def body():
                for g in range(CPC // GROUP):
                    ob = obp.tile([P, 2, GROUP, WO], _F32)  # [p, par, cc, j]
                    for cc in range(GROUP):
                        c = g * GROUP + cc
                        xt = xp.tile([P, 4, W], _F32)
                        nc.sync.dma_start(
                            out=xt[:],
                            in_=x[c].rearrange("(p r) w -> p r w", p=P),
                        )
                        rp = rpp.tile([P, 4, WO], _F32)
                        nc.vector.tensor_add(
                            rp[:],
                            xt[:, :, 0 : 2 * WO : 2],
                            xt[:, :, 1 : 2 * WO + 1 : 2],
                        )
                        nc.vector.tensor_add(
                            rp[:], rp[:], xt[:, :, 2 : 2 * WO + 2 : 2]
                        )
                        # even rows on GpSimd
                        ev = evp.tile([P, WO], _F32)
                        nc.gpsimd.tensor_add(ev[:], rp[:, 0, :], rp[:, 1, :])
                        nc.gpsimd.tensor_add(ev[:], ev[:], rp[:, 2, :])
                        # odd rows on PE: psum[m] = rp[m+1,0] + rp[m,2] + rp[m,3]
                        pst = psp.tile([P, WO], _F32)
                        nc.tensor.matmul(
                            pst[:], t_sb[:, 0, :], rp[:, 0, :],
                            start=True, stop=False,
                        )
                        nc.tensor.matmul(
                            pst[:], t_sb[:, 1, :], rp[:, 2, :],
                            start=False, stop=False,
                        )
                        nc.tensor.matmul(
                            pst[:], t_sb[:, 1, :], rp[:, 3, :],
                            start=False, stop=True,
                        )
                        # scale 1/9 on Act
                        nc.scalar.mul(ob[:, 0, cc, :], ev[:], 1.0 / 9.0)
                        nc.scalar.mul(ob[:, 1, cc, :], pst[:], 1.0 / 9.0)
                    og = out[g * GROUP : (g + 1) * GROUP]  # [GROUP, HO, WO]
                    nc.scalar.dma_start(
                        out=og[:, 0:HO:2, :].transpose([1, 0, 2]),
                        in_=ob[:, 0, :, :],
                    )
                    nc.scalar.dma_start(
                        out=og[:, 1:HO:2, :].transpose([1, 0, 2]),
                        in_=ob[0 : P - 1, 1, :, :],
                    )

            if repeat == 1:
                body()
            else:
                with tc.For_i(0, repeat, 1):
                    body()
    nc.compile()
    return nc


_NC_CACHE: dict = {}


def _get_nc(repeat: int = 1):
    if repeat not in _NC_CACHE:
        _NC_CACHE[repeat] = _build_nc(repeat)
    return _NC_CACHE[repeat]


def kernel(x: np.ndarray, **_unused) -> np.ndarray:
    assert x.shape == (B, C, H, W), x.shape
    x = np.ascontiguousarray(np.asarray(x, dtype=np.float32))
    tmat = _shift_mats()
    in_maps = [{"x": x[i], "tmat": tmat} for i in range(N_CORES)]
    res = run_bass_kernel_spmd(_get_nc(), in_maps, list(range(N_CORES)))
    return np.stack([res.results[i]["out"] for i in range(N_CORES)], axis=0)


# revision 4
# speedup vs baseline: 1.0928x; 1.0180x over previous
"""3x3 stride-2 VALID avg-pool over (8, 64, 512, 512) fp32 on 8 trn2 cores.

Sharding: data-parallel over batch — core i handles x[i] (64 planes of
512x512, contiguous 64 MiB slab). No communication.

Per-core dataflow (plane layout h = 4p + r, so output row 2p is entirely
within partition p and output row 2p+1 needs one row from partition p+1):
  1. DMA one plane (1 MiB, contiguous) into SBUF as [128p, 4r, 512w].
  2. DVE W-pool via strided views (2 tensor_adds over [128, 4, 255]):
       rp[p,r,j] = x[h,2j] + x[h,2j+1] + x[h,2j+2]
  3. GpSimd H-pool for even output rows (2 tensor_adds over [128, 255]):
       ev[p,j] = rp[p,0,j] + rp[p,1,j] + rp[p,2,j]        (= row 2p sum)
  4. PE H-pool for odd output rows (partition +1 shift is only legal on
     PE; 3 accumulating matmuls, N=255 fp32):
       psum[m,j] = rp[m+1,0,j] + rp[m,2,j] + rp[m,3,j]    (= row 2m+1 sum)
     with stationaries S (subdiagonal) and I (identity).
  5. Act scale by 1/9: ob_even <- ev (SBUF), ob_odd <- psum (PSUM).
  6. Output DMAs issued from the Act queue (keeps the SP queue free for
     input prefetch), batched 4 planes per pair of stores.
"""

import sys

sys.path.insert(0, "/opt/trn_rl_repo")

import numpy as np

from concourse import bacc, bass, mybir, tile
from concourse.bass_utils import run_bass_kernel_spmd

P = 128
B, C, H, W = 8, 64, 512, 512
KS, ST = 3, 2
HO = (H - KS) // ST + 1  # 255
WO = (W - KS) // ST + 1  # 255
CPC = C  # planes per core (one batch image per core)
GROUP = 4  # planes per output-DMA batch
N_CORES = 8

_F32 = mybir.dt.float32


def _shift_mats() -> np.ndarray:
    """T[:,0,:] = S (S[k,m]=1 iff k=m+1), T[:,1,:] = I."""
    t = np.zeros((P, 2, P), np.float32)
    k = np.arange(P)
    t[k[1:], 0, k[:-1]] = 1.0
    t[k, 1, k] = 1.0
    return t


def _build_nc(repeat: int = 1) -> bass.Bass:
    nc = bacc.Bacc(None)
    x = nc.declare_dram_parameter("x", [CPC, H, W], _F32, isOutput=False)
    tmat = nc.declare_dram_parameter("tmat", [P, 2, P], _F32, isOutput=False)
    out = nc.declare_dram_parameter("out", [CPC, HO, WO], _F32, isOutput=True)

    with tile.TileContext(nc) as tc:
        with (
            tc.tile_pool(name="const", bufs=1) as constp,
            tc.tile_pool(name="xin", bufs=4) as xp,
            tc.tile_pool(name="rp", bufs=4) as rpp,
            tc.tile_pool(name="ev", bufs=4) as evp,
            tc.tile_pool(name="ob", bufs=1) as obp,
            tc.tile_pool(name="ps", bufs=4, space="PSUM") as psp,
        ):
            # tmat load on the Act queue: keeps the SP queue free so the
            # first input DMA issues immediately.
            t_sb = constp.tile([P, 2, P], _F32)
            nc.scalar.dma_start(out=t_sb[:], in_=tmat[:])
            # all 64 scaled planes staged here until the input stream ends
            ob = obp.tile([P, 2, CPC, WO], _F32)  # [p, par, c, j]

            def body():
                for c in range(CPC):
                    xt = xp.tile([P, 4, W], _F32)
                    nc.sync.dma_start(
                        out=xt[:],
                        in_=x[c].rearrange("(p r) w -> p r w", p=P),
                    )
                    rp = rpp.tile([P, 4, WO], _F32)
                    nc.vector.tensor_add(
                        rp[:],
                        xt[:, :, 0 : 2 * WO : 2],
                        xt[:, :, 1 : 2 * WO + 1 : 2],
                    )
                    nc.vector.tensor_add(
                        rp[:], rp[:], xt[:, :, 2 : 2 * WO + 2 : 2]
                    )
                    # even rows on GpSimd
                    ev = evp.tile([P, WO], _F32)
                    nc.gpsimd.tensor_add(ev[:], rp[:, 0, :], rp[:, 1, :])
                    nc.gpsimd.tensor_add(ev[:], ev[:], rp[:, 2, :])
                    # odd rows on PE: psum[m] = rp[m+1,0] + rp[m,2] + rp[m,3]
                    pst = psp.tile([P, WO], _F32)
                    nc.tensor.matmul(
                        pst[:], t_sb[:, 0, :], rp[:, 0, :],
                        start=True, stop=False,
                    )
                    nc.tensor.matmul(
                        pst[:], t_sb[:, 1, :], rp[:, 2, :],
                        start=False, stop=False,
                    )
                    nc.tensor.matmul(
                        pst[:], t_sb[:, 1, :], rp[:, 3, :],
                        start=False, stop=True,
                    )
                    # scale 1/9 on Act
                    nc.scalar.mul(ob[:, 0, c, :], ev[:], 1.0 / 9.0)
                    nc.scalar.mul(ob[:, 1, c, :], pst[:], 1.0 / 9.0)
                # output DMAs after the whole input stream (same SP queue):
                # every chunk's compute is long done, so the stores pack the
                # DMA engines back-to-back through the drain window.
                for g in range(CPC // GROUP):
                    og = out[g * GROUP : (g + 1) * GROUP]  # [GROUP, HO, WO]
                    nc.sync.dma_start(
                        out=og[:, 0:HO:2, :].transpose([1, 0, 2]),
                        in_=ob[:, 0, g * GROUP : (g + 1) * GROUP, :],
                    )
                    nc.sync.dma_start(
                        out=og[:, 1:HO:2, :].transpose([1, 0, 2]),
                        in_=ob[0 : P - 1, 1, g * GROUP : (g + 1) * GROUP, :],
                    )

            if repeat == 1:
                body()
            else:
                with tc.For_i(0, repeat, 1):
                    body()
    nc.compile()
    return nc


_NC_CACHE: dict = {}


def _get_nc(repeat: int = 1):
    if repeat not in _NC_CACHE:
        _NC_CACHE[repeat] = _build_nc(repeat)
    return _NC_CACHE[repeat]


def kernel(x: np.ndarray, **_unused) -> np.ndarray:
    assert x.shape == (B, C, H, W), x.shape
    x = np.ascontiguousarray(np.asarray(x, dtype=np.float32))
    tmat = _shift_mats()
    in_maps = [{"x": x[i], "tmat": tmat} for i in range(N_CORES)]
    res = run_bass_kernel_spmd(_get_nc(), in_maps, list(range(N_CORES)))
    return np.stack([res.results[i]["out"] for i in range(N_CORES)], axis=0)


# revision 6
# speedup vs baseline: 1.0987x; 1.0054x over previous
"""3x3 stride-2 VALID avg-pool over (8, 64, 512, 512) fp32 on 8 trn2 cores.

Sharding: data-parallel over batch — core i handles x[i] (64 planes of
512x512, contiguous 64 MiB slab). No communication.

Per-core dataflow (plane layout h = 4p + r, so output row 2p is entirely
within partition p and output row 2p+1 needs one row from partition p+1):
  1. DMA one plane into SBUF as [128p, 4r, 511w] (row 511 / col 511 are
     outside every pooling window and are not transferred; the last
     partition's row slab is a separate 3-row DMA).
  2. DVE W-pool via strided views (2 tensor_adds over [128, 4, 255]):
       rp[p,r,j] = x[h,2j] + x[h,2j+1] + x[h,2j+2]
  3. GpSimd H-pool for even output rows (2 tensor_adds over [128, 255]):
       ev[p,j] = rp[p,0,j] + rp[p,1,j] + rp[p,2,j]        (= row 2p sum)
  4. PE H-pool for odd output rows (partition +1 shift is only legal on
     PE; 3 accumulating matmuls, N=255 fp32):
       psum[m,j] = rp[m+1,0,j] + rp[m,2,j] + rp[m,3,j]    (= row 2m+1 sum)
     with stationaries S (subdiagonal) and I (identity), built on-chip
     from one GpSimd iota + one DVE compare (no DMA).
  5. Act scale by 1/9: ob_even <- ev (SBUF), ob_odd <- psum (PSUM), into
     a persistent [128, 2, 64, 255] staging tile.
  6. All output DMAs issue on the SP queue after the whole input stream:
     every store's compute is long done, so the stores pack the DMA
     engines back-to-back through the drain window.
"""

import sys

sys.path.insert(0, "/opt/trn_rl_repo")

import numpy as np

from concourse import bacc, bass, mybir, tile
from concourse.bass_utils import run_bass_kernel_spmd

P = 128
B, C, H, W = 8, 64, 512, 512
KS, ST = 3, 2
HO = (H - KS) // ST + 1  # 255
WO = (W - KS) // ST + 1  # 255
WU = 2 * WO + 1  # 511 columns actually read
CPC = C  # planes per core (one batch image per core)
GROUP = 4  # planes per output-DMA batch
N_CORES = 8

_F32 = mybir.dt.float32
_I32 = mybir.dt.int32


def _build_nc(repeat: int = 1) -> bass.Bass:
    nc = bacc.Bacc(None)
    x = nc.declare_dram_parameter("x", [CPC, H, W], _F32, isOutput=False)
    out = nc.declare_dram_parameter("out", [CPC, HO, WO], _F32, isOutput=True)

    with tile.TileContext(nc) as tc:
        with (
            tc.tile_pool(name="const", bufs=1) as constp,
            tc.tile_pool(name="xin", bufs=4) as xp,
            tc.tile_pool(name="rp", bufs=4) as rpp,
            tc.tile_pool(name="ev", bufs=4) as evp,
            tc.tile_pool(name="ob", bufs=1) as obp,
            tc.tile_pool(name="ps", bufs=4, space="PSUM") as psp,
        ):
            # t_sb[:,0,:] = S (S[k,m]=1 iff k=m+1), t_sb[:,1,:] = I.
            # value(k,t,m) = -t + m - k == -1 iff (t=0, m=k-1) or (t=1, m=k)
            it = constp.tile([P, 2, P], _I32)
            nc.gpsimd.iota(
                it[:], pattern=[[-1, 2], [1, P]], base=0, channel_multiplier=-1
            )
            t_sb = constp.tile([P, 2, P], _F32)
            nc.vector.tensor_scalar(
                t_sb[:], it[:], -1, None, mybir.AluOpType.is_equal
            )
            # all 64 scaled planes staged here until the input stream ends
            ob = obp.tile([P, 2, CPC, WO], _F32)  # [p, par, c, j]

            def body():
                for c in range(CPC):
                    xt = xp.tile([P, 4, WU], _F32)
                    nc.sync.dma_start(
                        out=xt[0 : P - 1, :, :],
                        in_=x[c][0 : 4 * (P - 1), 0:WU].rearrange(
                            "(p r) w -> p r w", p=P - 1
                        ),
                    )
                    nc.sync.dma_start(
                        out=xt[P - 1 : P, 0:3, :],
                        in_=x[c][4 * (P - 1) : 4 * P - 1, 0:WU].rearrange(
                            "(p r) w -> p r w", p=1
                        ),
                    )
                    rp = rpp.tile([P, 4, WO], _F32)
                    nc.vector.tensor_add(
                        rp[:],
                        xt[:, :, 0 : 2 * WO : 2],
                        xt[:, :, 1 : 2 * WO + 1 : 2],
                    )
                    nc.vector.tensor_add(rp[:], rp[:], xt[:, :, 2:WU:2])
                    # even rows on GpSimd
                    ev = evp.tile([P, WO], _F32)
                    nc.gpsimd.tensor_add(ev[:], rp[:, 0, :], rp[:, 1, :])
                    nc.gpsimd.tensor_add(ev[:], ev[:], rp[:, 2, :])
                    # odd rows on PE: psum[m] = rp[m+1,0] + rp[m,2] + rp[m,3]
                    # (last matmul contracts K=127 only: rp[127,3] is row 511,
                    # which is never loaded)
                    pst = psp.tile([P, WO], _F32)
                    nc.tensor.matmul(
                        pst[:], t_sb[:, 0, :], rp[:, 0, :],
                        start=True, stop=False,
                    )
                    nc.tensor.matmul(
                        pst[:], t_sb[:, 1, :], rp[:, 2, :],
                        start=False, stop=False,
                    )
                    nc.tensor.matmul(
                        pst[:], t_sb[0 : P - 1, 1, :], rp[0 : P - 1, 3, :],
                        start=False, stop=True,
                    )
                    # scale 1/9 on Act
                    nc.scalar.mul(ob[:, 0, c, :], ev[:], 1.0 / 9.0)
                    nc.scalar.mul(ob[:, 1, c, :], pst[:], 1.0 / 9.0)
                # output DMAs after the whole input stream (same SP queue)
                for g in range(CPC // GROUP):
                    og = out[g * GROUP : (g + 1) * GROUP]  # [GROUP, HO, WO]
                    nc.sync.dma_start(
                        out=og[:, 0:HO:2, :].transpose([1, 0, 2]),
                        in_=ob[:, 0, g * GROUP : (g + 1) * GROUP, :],
                    )
                    nc.sync.dma_start(
                        out=og[:, 1:HO:2, :].transpose([1, 0, 2]),
                        in_=ob[0 : P - 1, 1, g * GROUP : (g + 1) * GROUP, :],
                    )

            if repeat == 1:
                body()
            else:
                with tc.For_i(0, repeat, 1):
                    body()
    nc.compile()
    return nc


_NC_CACHE: dict = {}


def _get_nc(repeat: int = 1):
    if repeat not in _NC_CACHE:
        _NC_CACHE[repeat] = _build_nc(repeat)
    return _NC_CACHE[repeat]


def make_in_maps(x: np.ndarray) -> list[dict]:
    return [{"x": np.ascontiguousarray(x[i])} for i in range(N_CORES)]


def kernel(x: np.ndarray, **_unused) -> np.ndarray:
    assert x.shape == (B, C, H, W), x.shape
    x = np.asarray(x, dtype=np.float32)
    res = run_bass_kernel_spmd(_get_nc(), make_in_maps(x), list(range(N_CORES)))
    return np.stack([res.results[i]["out"] for i in range(N_CORES)], axis=0)
